# revision 28
# baseline (speedup 1.0000x reference)
"""Fused QKV+RoPE+GQA-attention kernel for Trainium2, sharded over 8 NeuronCores.

Sharding: data-parallel over batch (B=2), tensor-parallel over heads (4 groups of
8 q-heads / 2 kv-heads per batch element).  Each core computes its output slice
out[b, :, g*1024:(g+1)*1024] independently; no collectives.

Per-core pipeline (n=2048 seq, c=4096 model, d=128 head dim), all matmul
operands in bf16 (fp32 PSUM accumulation; rel-err budget 2e-2 leaves ~20x room):
  - Weights live resident in SBUF (12.6 MB bf16, loaded once) — the fp32r
    predecessor re-streamed all 25 MB of weights from HBM every chunk.
  - QKV^T projection:  Q^T/K^T = W X^T (feature-major, head_dim on partitions),
    V = X W^T (position-major).  bf16 streams 1 col/cycle at any moving dim and
    FWL halves the LDWEIGHTS cost.
  - RoPE without partition shifts: rot_half(q) = sign * (R q) where R is the
    swap-halves permutation applied via one extra matmul per head-chunk, and the
    sign is folded into the sin table on host.
  - Attention: S^T = K_rope Q_rope^T (k on partitions, q on free dim), causal
    structure exploited at 128-tile granularity, the real attention-mask applied
    only inside diagonal 128x128 blocks, softmax without max-subtraction
    (logits are O(10); exp(-3.4e38) = 0 exactly on the ACT LUT), denominator
    from a ones column augmented into V (AV moving dim = 129, per-kv-head
    slice), per-s output drained on the Vector engine as soon as its causal
    depth completes, per-(head,s) 128x128 output DMA on the Sync queue.
  - Emission interleaves attention(chunk i-1) with projection(chunk i): the PE
    executes its queue in order, so alternating independent streams fills each
    stream's dependency bubbles and keeps the HAM clock gate warm.
"""

from contextlib import ExitStack

import numpy as np
import ml_dtypes

import concourse.tile as tile
from concourse import bacc, mybir
from concourse.bass_utils import run_bass_kernel_spmd

F32 = mybir.dt.float32
BF16 = mybir.dt.bfloat16

B, N, C = 2, 2048, 4096
NUM_HEADS, KV_HEADS, HEAD_DIM = 32, 8, 128
GROUPS = 4                     # head groups per batch element
QH = NUM_HEADS // GROUPS       # 8 q heads per core
KVH = KV_HEADS // GROUPS       # 2 kv heads per core
N_CORES = B * GROUPS

NCHUNK = 512                   # seq positions per pass
NCHUNKS = N // NCHUNK          # 4
CC = C // 128                  # 32 contraction chunks
NT = N // 128                  # 16 position tiles
VW = 129                       # AV block width: [v(128) | ones(1)]


def _build_program():
    nc = bacc.Bacc("TRN2", target_bir_lowering=False, debug=False, num_devices=N_CORES)

    xt = nc.dram_tensor("xt", [NCHUNKS, 8, 128, 4, NCHUNK], BF16, kind="ExternalInput").ap()
    wt = nc.dram_tensor("wt", [128, 6, 8, 4, 256], BF16, kind="ExternalInput").ap()
    cosT = nc.dram_tensor("cosT", [128, N], F32, kind="ExternalInput").ap()
    sinmT = nc.dram_tensor("sinmT", [128, N], F32, kind="ExternalInput").ap()
    maskd = nc.dram_tensor("maskd", [NCHUNKS, 128, 4, 128], BF16, kind="ExternalInput").ap()
    obs = nc.dram_tensor("obs", [128, NT], F32, kind="ExternalInput").ap()
    rmat = nc.dram_tensor("rmat", [128, 128], BF16, kind="ExternalInput").ap()
    out = nc.dram_tensor("out", [N, QH * 128], F32, kind="ExternalOutput").ap()

    with tile.TileContext(nc) as tc, ExitStack() as ctx:
        singles = ctx.enter_context(tc.tile_pool(name="singles", bufs=1))
        xpool = ctx.enter_context(tc.tile_pool(name="xt", bufs=9))
        rpool = ctx.enter_context(tc.tile_pool(name="rope", bufs=2))
        cspool = ctx.enter_context(tc.tile_pool(name="cs", bufs=2))
        qtpool = ctx.enter_context(tc.tile_pool(name="qt", bufs=2))
        ptpool = ctx.enter_context(tc.tile_pool(name="pt", bufs=6))
        mpool = ctx.enter_context(tc.tile_pool(name="mask", bufs=2))
        opool = ctx.enter_context(tc.tile_pool(name="outp", bufs=6))
        spool = ctx.enter_context(tc.tile_pool(name="small", bufs=8))
        pp_proj = ctx.enter_context(tc.tile_pool(name="pp_proj", bufs=3, space="PSUM"))
        pp_misc = ctx.enter_context(tc.tile_pool(name="pp_misc", bufs=3, space="PSUM"))
        pp_av = ctx.enter_context(tc.tile_pool(name="pp_av", bufs=2, space="PSUM"))

        # ---- persistent tiles ----
        wt_sb = singles.tile([128, 6, 8, 4, 256], BF16, tag="wt")
        rmat_sb = singles.tile([128, 128], BF16, tag="rmat")
        obs_sb = singles.tile([128, NT], F32, tag="obs")
        kt_res = singles.tile([128, KVH, N], BF16, tag="ktres")      # K^T rope'd
        vaug = singles.tile([128, NT, KVH, VW], BF16, tag="vaug")    # V + ones col

        wscr = singles.tile([128, NCHUNK], BF16, tag="wscr")

        def emit_startup():
            """Chunk-0 + persistent-tile DMAs, deadline-ordered, round-robin
            over all 3 DMA-capable queues (~125 GB/s each).  Interleaving x
            tiles with weight pieces keeps every queue delivering bytes that
            chunk-0 projection (K/V first, then q-fgs) is about to consume."""
            st = dict(n0=0)
            xsub = []
            for j in range(8):
                t = xpool.tile([128, 4, NCHUNK], BF16, tag="xt", name=f"xt0_{j}")
                xsub.append(t)
            st["xsub"] = xsub
            cos_c = cspool.tile([128, NCHUNK], F32, tag="cos", name="cos0")
            sinm_c = cspool.tile([128, NCHUNK], F32, tag="sinm", name="sinm0")
            st["cos"], st["sinm"] = cos_c, sinm_c
            st["qt"] = qtpool.tile([128, QH, NCHUNK], BF16, tag="qt", name="qt0")

            engs = [nc.gpsimd, nc.sync, nc.scalar]
            kq = [0]

            def dma(out, in_):
                engs[kq[0] % 3].dma_start(out=out, in_=in_)
                kq[0] += 1

            def wpiece(fg, c0, w):
                dma(wt_sb[:, fg, c0:c0 + w], wt[:, fg, c0:c0 + w])

            for j in range(4):                       # xt0-3 / fg4 quarters
                dma(xsub[j], xt[0, j])
                wpiece(4, 2 * j, 2)
            dma(xsub[4], xt[0, 4])
            dma(xsub[5], xt[0, 5])
            dma(xsub[6], xt[0, 6])
            dma(rmat_sb, rmat)
            dma(cos_c, cosT[:, 0:NCHUNK])
            dma(xsub[7], xt[0, 7])
            dma(sinm_c, sinmT[:, 0:NCHUNK])
            for j in range(4):                       # fg5 (V) quarters
                wpiece(5, 2 * j, 2)
            dma(obs_sb, obs)
            for fg in range(4):                      # q-fg halves, in order
                wpiece(fg, 0, 4)
                wpiece(fg, 4, 4)

            nc.vector.memset(vaug[:, :, :, 128:129], 1.0)
            # HAM warm-up: dummy matmuls on a memset scratch run during the
            # DMA dead zone, so real matmuls start at 2.4GHz with no deps
            nc.vector.memset(wscr, 0.0)
            for k in range(16):
                pw = pp_misc.tile([128, NCHUNK], F32, tag="misc", name=f"warm{k}")
                nc.tensor.matmul(pw, wscr[:, 0:128], wscr, start=True, stop=True)
            return st

        def rope_pair(ps_a, ps_b, dest_a, dest_b, cos_c, sinm_c):
            """dest = ps*cos + (R @ ps)*sinm for two head chunks."""
            qq_a = rpool.tile([128, NCHUNK], BF16, tag="qq")
            nc.scalar.copy(qq_a, ps_a)
            qq_b = rpool.tile([128, NCHUNK], BF16, tag="qq2")
            nc.scalar.copy(qq_b, ps_b)
            pr_a = pp_misc.tile([128, NCHUNK], F32, tag="misc")
            nc.tensor.matmul(pr_a, rmat_sb, qq_a, start=True, stop=True)
            pr_b = pp_misc.tile([128, NCHUNK], F32, tag="misc")
            nc.tensor.matmul(pr_b, rmat_sb, qq_b, start=True, stop=True)
            for ps, pr, qq, dest, tg in ((ps_a, pr_a, qq_a, dest_a, "a"),
                                         (ps_b, pr_b, qq_b, dest_b, "b")):
                tcs = rpool.tile([128, NCHUNK], BF16, tag="tcs" + tg, bufs=1)
                nc.vector.tensor_mul(tcs, ps, cos_c)
                # qq is dead after the R-matmul; reuse its slot for the sin product
                nc.vector.tensor_mul(qq, pr, sinm_c)
                nc.vector.tensor_add(dest, tcs, qq)

        def chunk_dmas(qc):
            """Emit the input DMAs for chunk qc; returns chunk state."""
            n0 = qc * NCHUNK
            xsub = []
            for j in range(8):
                t = xpool.tile([128, 4, NCHUNK], BF16, tag="xt", name=f"xt{qc}_{j}")
                nc.gpsimd.dma_start(out=t, in_=xt[qc, j])
                xsub.append(t)
            cos_c = cspool.tile([128, NCHUNK], F32, tag="cos", name=f"cos{qc}")
            nc.gpsimd.dma_start(out=cos_c, in_=cosT[:, n0:n0 + NCHUNK])
            sinm_c = cspool.tile([128, NCHUNK], F32, tag="sinm", name=f"sinm{qc}")
            nc.gpsimd.dma_start(out=sinm_c, in_=sinmT[:, n0:n0 + NCHUNK])
            qt_chunk = qtpool.tile([128, QH, NCHUNK], BF16, tag="qt", name=f"qt{qc}")
            return dict(n0=n0, xsub=xsub, cos=cos_c, sinm=sinm_c, qt=qt_chunk)

        def proj_units(qc, st):
            """13 emission units: 5 feature-group c-loops (+deferred rope), 2 V passes."""
            n0, xsub = st["n0"], st["xsub"]

            def xc(c):
                return xsub[c // 4][:, c % 4, :]

            pend = {}

            def emit_fg(fg, half):
                if half == 0:
                    pend[fg] = (
                        pp_proj.tile([128, NCHUNK], F32, tag="proj", name=f"pa{qc}_{fg}"),
                        pp_proj.tile([128, NCHUNK], F32, tag="proj", name=f"pb{qc}_{fg}"))
                ps_a, ps_b = pend[fg]
                # a/b alternated at cq granularity: the first 4 a-matmuls give
                # the previous fg's rope ~1.7us to drain its banks (bufs=3),
                # and each weight piece is consumed at half the a-then-b rate,
                # which keeps chunk-0 projection behind the HBM delivery curve
                for cq in range(4 * half, 4 * half + 4):
                    for ps, f0 in ((ps_a, 0), (ps_b, 128)):
                        for i in range(4):
                            c = 4 * cq + i
                            nc.tensor.matmul(ps, wt_sb[:, fg, cq, i, f0:f0 + 128],
                                             xc(c), start=(c == 0), stop=(c == CC - 1))

            def emit_rope(fg):
                ps_a, ps_b = pend.pop(fg)
                if fg < 4:
                    d_a = st["qt"][:, 2 * fg, :]
                    d_b = st["qt"][:, 2 * fg + 1, :]
                else:
                    d_a = kt_res[:, 0, n0:n0 + NCHUNK]
                    d_b = kt_res[:, 1, n0:n0 + NCHUNK]
                rope_pair(ps_a, ps_b, d_a, d_b, st["cos"], st["sinm"])

            vps = {}

            def emit_vhalf(p, half):
                if half == 0:
                    vps[p] = [pp_proj.tile([128, 256], F32, tag="proj",
                                           name=f"pv{qc}_{p}_{i}") for i in range(2)]
                ps = vps[p]
                # k outer: ps[1]'s first matmul comes 1.7us after ps[0]'s,
                # giving the previous unit's banks time to drain
                for k in range(2):
                    ns = 2 * p + k
                    for cq in range(4 * half, 4 * half + 4):
                        for i in range(4):
                            c = 4 * cq + i
                            nc.tensor.matmul(ps[k], xc(c)[:, 128 * ns:128 * ns + 128],
                                             wt_sb[:, 5, cq, i, :],
                                             start=(c == 0), stop=(c == CC - 1))
                if half == 1:
                    for k in range(2):
                        ns = 2 * p + k
                        nt_i = 4 * qc + ns
                        nc.scalar.copy(vaug[:, nt_i, :, 0:128],
                                       ps[k].rearrange("p (h w) -> p h w", h=2))

            # K/V first, then the q feature-groups with their ropes streaming
            # out progressively — on the last chunk this lets attention heads
            # interleave into the projection as soon as rope(h//2) lands
            units = [lambda: emit_fg(4, 0), lambda: emit_fg(4, 1),
                     lambda: (emit_rope(4), emit_vhalf(0, 0)),
                     lambda: emit_vhalf(0, 1),
                     lambda: emit_vhalf(1, 0),
                     lambda: emit_vhalf(1, 1),
                     lambda: emit_fg(0, 0), lambda: emit_fg(0, 1)]
            for fg in range(1, 4):
                units.append(lambda fg=fg: (emit_rope(fg - 1), emit_fg(fg, 0)))
                units.append(lambda fg=fg: emit_fg(fg, 1))
            units.append(lambda: emit_rope(3))
            return units

        def attn_units(qc, st):
            """9 emission units: mask DMA + 8 heads; per-s drain as soon as done."""
            n0, qt_chunk = st["n0"], st["qt"]
            nk = 4 * qc + 4
            shared = {}

            def emit_pre():
                m_sb = mpool.tile([128, 4, 128], BF16, tag="mask", name=f"m{qc}")
                nc.gpsimd.dma_start(out=m_sb, in_=maskd[qc])
                shared["m"] = m_sb

            def emit_head(h):
                kv = h // (QH // KVH)
                m_sb = shared["m"]
                pt_tiles = {}
                STAG = 3

                def qk_step(kt):
                    # columns left of q-sub kd are never consumed (per-s AV depth)
                    kd = kt - 4 * qc
                    lo = 128 * kd if kd > 0 else 0
                    ps_s = pp_misc.tile([128, NCHUNK], F32, tag="misc",
                                        name=f"s{qc}_{h}_{kt}")
                    nc.tensor.matmul(ps_s[:, lo:], kt_res[:, kv, 128 * kt:128 * kt + 128],
                                     qt_chunk[:, h, lo:], start=True, stop=True)
                    pt = ptpool.tile([128, NCHUNK], BF16, tag="pt", name=f"pt{qc}_{h}_{kt}")
                    nc.scalar.activation(out=pt[:, lo:], in_=ps_s[:, lo:],
                                         func=mybir.ActivationFunctionType.Exp)
                    if kd >= 0:
                        # causal cut inside the diagonal 128x128 block as a 0/1
                        # multiply on exp'd bf16 in SBUF — off the QK->exp
                        # chain, and the ones-column denominator stays exact
                        # since it accumulates from the zeroed pt
                        nc.vector.tensor_mul(pt[:, 128 * kd:128 * kd + 128],
                                             pt[:, 128 * kd:128 * kd + 128],
                                             m_sb[:, kd, :])
                    pt_tiles[kt] = pt

                # two q-subtiles share one PSUM bank (129-col groups at 0 and
                # 256): accumulation runs start=False onto a DVE-zeroed bank,
                # so the whole-bank has_written clear of start=True never fires
                av_bank = [pp_av.tile([128, 512], F32, tag="av",
                                      name=f"o{qc}_{h}_{p}") for p in range(2)]
                for p in range(2):
                    nc.vector.memset(av_bank[p][:, 0:256 + VW], 0.0)

                def ps_os(s):
                    return av_bank[s // 2][:, 256 * (s % 2):256 * (s % 2) + VW]

                def drain_s(s):
                    po = ps_os(s)
                    den = spool.tile([128, 1], F32, tag="den")
                    nc.vector.reciprocal(den, po[:, 128:129])
                    sc = spool.tile([128, 1], F32, tag="sc")
                    nc.vector.tensor_mul(sc, den, obs_sb[:, 4 * qc + s:4 * qc + s + 1])
                    ob = opool.tile([128, 128], F32, tag="ob", name=f"ob{qc}_{h}_{s}")
                    nc.vector.tensor_scalar_mul(ob, po[:, 0:128], sc)
                    nc.sync.dma_start(out=out[n0 + 128 * s:n0 + 128 * (s + 1),
                                              128 * h:128 * (h + 1)], in_=ob)

                def av_step(kt):
                    pt = pt_tiles.pop(kt)
                    for s in range(4):
                        if kt > 4 * qc + s:
                            continue  # beyond this q-sub's causal depth
                        nc.tensor.matmul(ps_os(s), pt[:, 128 * s:128 * s + 128],
                                         vaug[:, kt, kv, :], start=False,
                                         stop=(kt == 4 * qc + s),
                                         skip_group_check=True)
                    # drain a bank pair only once BOTH its subtiles stopped:
                    # a drain of s0 while s1 still accumulates would force a
                    # PE-behind-DVE serialization on the shared bank
                    if kt == 4 * qc + 1:
                        drain_s(0), drain_s(1)
                    elif kt == 4 * qc + 3:
                        drain_s(2), drain_s(3)

                for kt in range(nk + STAG):
                    if kt < nk:
                        qk_step(kt)
                    if kt >= STAG:
                        av_step(kt - STAG)

            units = [emit_pre]
            for h in range(QH):
                units.append(lambda h=h: emit_head(h))
            return units

        # ---- pipelined emission: attention(qc) heads follow proj(qc)'s ropes
        # inside the same round; the last head pair's exp backlog drains under
        # round qc+1's K/V projection, so ACT never piles up at the end ----
        states = {}
        for qc in range(NCHUNKS):
            states[qc] = emit_startup() if qc == 0 else chunk_dmas(qc)
            pu = proj_units(qc, states[qc])
            au = attn_units(qc, states[qc])
            au[0]()                              # mask DMA up front
            for u in pu[0:9]:                    # K/V, V-copies, fg0, rope0
                u()
            for u in [au[1], pu[9], au[2], pu[10], au[3], pu[11], au[4],
                      pu[12], au[5], pu[13], au[6], pu[14], au[7], au[8]]:
                u()
            if qc > 0:
                del states[qc - 1]

    nc.compile()
    return nc


_NC_CACHE = None


def _get_program():
    global _NC_CACHE
    if _NC_CACHE is None:
        _NC_CACHE = _build_program()
    return _NC_CACHE


def _prep_core_inputs(input, weight, cos_cached, sin_cached, attention_mask,
                      position_ids, observation_mask):
    """Build the 8 per-core input maps (host-side shard + layout + bf16 cast)."""
    bf16 = ml_dtypes.bfloat16
    input = np.asarray(input, dtype=np.float32)
    weight = np.asarray(weight, dtype=np.float32)
    cos_cached = np.asarray(cos_cached, dtype=np.float32)
    sin_cached = np.asarray(sin_cached, dtype=np.float32)
    attention_mask = np.asarray(attention_mask, dtype=np.float32)
    position_ids = np.asarray(position_ids)
    observation_mask = np.asarray(observation_mask)

    scale = 1.0 / np.sqrt(HEAD_DIM)
    rmat = np.zeros((128, 128), dtype=np.float32)
    idx = np.arange(128)
    rmat[idx, (idx + 64) % 128] = 1.0
    rmat = rmat.astype(bf16)

    in_maps = []
    for core in range(N_CORES):
        b, g = core // GROUPS, core % GROUPS
        xtT = input[b].T.astype(bf16)                                  # [C, N]
        xt = np.ascontiguousarray(
            xtT.reshape(8, 4, 128, NCHUNKS, NCHUNK).transpose(3, 0, 2, 1, 4))

        wq = weight[g * QH * 128:(g + 1) * QH * 128] * scale           # [1024, C]
        k_off = NUM_HEADS * 128
        wk = weight[k_off + g * KVH * 128:k_off + (g + 1) * KVH * 128]  # [256, C]
        v_off = k_off + KV_HEADS * 128
        wv = weight[v_off + g * KVH * 128:v_off + (g + 1) * KVH * 128]  # [256, C]
        wtT = np.concatenate([wq, wk, wv], axis=0).T.astype(bf16)      # [C, 1536]
        wt = np.ascontiguousarray(
            wtT.reshape(8, 4, 128, 6, 256).transpose(2, 3, 0, 1, 4))   # [128,6,8,4,256]

        pos = position_ids[b]
        cosT = np.ascontiguousarray(cos_cached[0, 0][pos].T)           # [128, N]
        sinmT = np.ascontiguousarray(sin_cached[0, 0][pos].T)
        sinmT[:64] = -sinmT[:64]

        m = attention_mask[b, 0]                                       # [N, N]
        maskd = np.stack([
            np.stack([m[t * 128:(t + 1) * 128, t * 128:(t + 1) * 128].T
                      for t in range(4 * qc, 4 * qc + 4)]).transpose(1, 0, 2)
            for qc in range(NCHUNKS)])                                 # [4, 128, 4, 128]
        maskd = np.ascontiguousarray((maskd == 0.0).astype(bf16))      # 0/1 keep-mask

        obsf = np.ascontiguousarray(
            (observation_mask[b] == 1).astype(np.float32).reshape(NT, 128).T)

        in_maps.append(dict(xt=xt, wt=wt, cosT=cosT, sinmT=sinmT, maskd=maskd,
                            obs=obsf, rmat=rmat))
    return in_maps


def run(inputs: dict, trace: bool = False):
    """Run the sharded kernel; returns (full_output [B*N, C] fp32, BassKernelResults)."""
    nc = _get_program()
    in_maps = _prep_core_inputs(**inputs)
    res = run_bass_kernel_spmd(nc, in_maps, core_ids=list(range(N_CORES)), trace=trace)
    full = np.empty((B, N, C), dtype=np.float32)
    for core in range(N_CORES):
        b, g = core // GROUPS, core % GROUPS
        full[b, :, g * QH * 128:(g + 1) * QH * 128] = res.results[core]["out"]
    return full.reshape(B * N, C), res


def kernel(**inputs) -> np.ndarray:
    out, _ = run(inputs)
    return out


# revision 31
# speedup vs baseline: 1.0088x; 1.0088x over previous
"""Fused QKV+RoPE+GQA-attention kernel for Trainium2, sharded over 8 NeuronCores.

Sharding: data-parallel over batch (B=2), tensor-parallel over heads (4 groups of
8 q-heads / 2 kv-heads per batch element).  Each core computes its output slice
out[b, :, g*1024:(g+1)*1024] independently; no collectives.

Per-core pipeline (n=2048 seq, c=4096 model, d=128 head dim), all matmul
operands in bf16 (fp32 PSUM accumulation; rel-err budget 2e-2 leaves ~20x room):
  - Weights live resident in SBUF (12.6 MB bf16, loaded once) — the fp32r
    predecessor re-streamed all 25 MB of weights from HBM every chunk.
  - QKV^T projection:  Q^T/K^T = W X^T (feature-major, head_dim on partitions),
    V = X W^T (position-major).  bf16 streams 1 col/cycle at any moving dim and
    FWL halves the LDWEIGHTS cost.
  - RoPE without partition shifts: rot_half(q) = sign * (R q) where R is the
    swap-halves permutation applied via one extra matmul per head-chunk, and the
    sign is folded into the sin table on host.
  - Attention: S^T = K_rope Q_rope^T (k on partitions, q on free dim), causal
    structure exploited at 128-tile granularity, the real attention-mask applied
    only inside diagonal 128x128 blocks, softmax without max-subtraction
    (logits are O(10); exp(-3.4e38) = 0 exactly on the ACT LUT), denominator
    from a ones column augmented into V (AV moving dim = 129, per-kv-head
    slice), per-s output drained on the Vector engine as soon as its causal
    depth completes, per-(head,s) 128x128 output DMA on the Sync queue.
  - Emission interleaves attention(chunk i-1) with projection(chunk i): the PE
    executes its queue in order, so alternating independent streams fills each
    stream's dependency bubbles and keeps the HAM clock gate warm.
"""

from contextlib import ExitStack

import numpy as np
import ml_dtypes

import concourse.tile as tile
from concourse import bacc, mybir
from concourse.bass_utils import run_bass_kernel_spmd

F32 = mybir.dt.float32
BF16 = mybir.dt.bfloat16

B, N, C = 2, 2048, 4096
NUM_HEADS, KV_HEADS, HEAD_DIM = 32, 8, 128
GROUPS = 4                     # head groups per batch element
QH = NUM_HEADS // GROUPS       # 8 q heads per core
KVH = KV_HEADS // GROUPS       # 2 kv heads per core
N_CORES = B * GROUPS

NCHUNK = 512                   # seq positions per pass
NCHUNKS = N // NCHUNK          # 4
CC = C // 128                  # 32 contraction chunks
NT = N // 128                  # 16 position tiles
VW = 129                       # AV block width: [v(128) | ones(1)]


def _build_program():
    nc = bacc.Bacc("TRN2", target_bir_lowering=False, debug=False, num_devices=N_CORES)

    xt = nc.dram_tensor("xt", [NCHUNKS, 8, 128, 4, NCHUNK], BF16, kind="ExternalInput").ap()
    wt = nc.dram_tensor("wt", [128, 6, 8, 4, 256], BF16, kind="ExternalInput").ap()
    cosT = nc.dram_tensor("cosT", [128, N], F32, kind="ExternalInput").ap()
    sinmT = nc.dram_tensor("sinmT", [128, N], F32, kind="ExternalInput").ap()
    maskd = nc.dram_tensor("maskd", [NCHUNKS, 128, 4, 128], BF16, kind="ExternalInput").ap()
    obs = nc.dram_tensor("obs", [128, NT], F32, kind="ExternalInput").ap()
    rmat = nc.dram_tensor("rmat", [128, 128], BF16, kind="ExternalInput").ap()
    out = nc.dram_tensor("out", [N, QH * 128], F32, kind="ExternalOutput").ap()

    with tile.TileContext(nc) as tc, ExitStack() as ctx:
        singles = ctx.enter_context(tc.tile_pool(name="singles", bufs=1))
        xpool = ctx.enter_context(tc.tile_pool(name="xt", bufs=9))
        rpool = ctx.enter_context(tc.tile_pool(name="rope", bufs=2))
        cspool = ctx.enter_context(tc.tile_pool(name="cs", bufs=2))
        qtpool = ctx.enter_context(tc.tile_pool(name="qt", bufs=2))
        ptpool = ctx.enter_context(tc.tile_pool(name="pt", bufs=6))
        mpool = ctx.enter_context(tc.tile_pool(name="mask", bufs=2))
        opool = ctx.enter_context(tc.tile_pool(name="outp", bufs=6))
        spool = ctx.enter_context(tc.tile_pool(name="small", bufs=8))
        pp_proj = ctx.enter_context(tc.tile_pool(name="pp_proj", bufs=3, space="PSUM"))
        pp_misc = ctx.enter_context(tc.tile_pool(name="pp_misc", bufs=3, space="PSUM"))
        pp_av = ctx.enter_context(tc.tile_pool(name="pp_av", bufs=2, space="PSUM"))

        # ---- persistent tiles ----
        wt_sb = singles.tile([128, 6, 8, 4, 256], BF16, tag="wt")
        rmat_sb = singles.tile([128, 128], BF16, tag="rmat")
        obs_sb = singles.tile([128, NT], F32, tag="obs")
        kt_res = singles.tile([128, KVH, N], BF16, tag="ktres")      # K^T rope'd
        vaug = singles.tile([128, NT, KVH, VW], BF16, tag="vaug")    # V + ones col

        wscr = singles.tile([128, NCHUNK], BF16, tag="wscr")

        def emit_startup():
            """Chunk-0 + persistent-tile DMAs, deadline-ordered, round-robin
            over all 3 DMA-capable queues (~125 GB/s each).  Interleaving x
            tiles with weight pieces keeps every queue delivering bytes that
            chunk-0 projection (K/V first, then q-fgs) is about to consume."""
            st = dict(n0=0)
            xsub = []
            for j in range(8):
                t = xpool.tile([128, 4, NCHUNK], BF16, tag="xt", name=f"xt0_{j}")
                xsub.append(t)
            st["xsub"] = xsub
            cos_c = cspool.tile([128, NCHUNK], F32, tag="cos", name="cos0")
            sinm_c = cspool.tile([128, NCHUNK], F32, tag="sinm", name="sinm0")
            st["cos"], st["sinm"] = cos_c, sinm_c
            st["qt"] = qtpool.tile([128, QH, NCHUNK], BF16, tag="qt", name="qt0")

            engs = [nc.gpsimd, nc.sync, nc.scalar]
            kq = [0]

            def dma(out, in_):
                engs[kq[0] % 3].dma_start(out=out, in_=in_)
                kq[0] += 1

            def wpiece(fg, c0, w):
                dma(wt_sb[:, fg, c0:c0 + w], wt[:, fg, c0:c0 + w])

            for j in range(4):                       # xt0-3 / fg4 quarters
                dma(xsub[j], xt[0, j])
                wpiece(4, 2 * j, 2)
            dma(xsub[4], xt[0, 4])
            dma(xsub[5], xt[0, 5])
            dma(xsub[6], xt[0, 6])
            dma(rmat_sb, rmat)
            dma(cos_c, cosT[:, 0:NCHUNK])
            dma(xsub[7], xt[0, 7])
            dma(sinm_c, sinmT[:, 0:NCHUNK])
            for j in range(4):                       # fg5 (V) quarters
                wpiece(5, 2 * j, 2)
            dma(obs_sb, obs)
            for fg in range(4):                      # q-fg halves, in order
                wpiece(fg, 0, 4)
                wpiece(fg, 4, 4)

            nc.vector.memset(vaug[:, :, :, 128:129], 1.0)
            # HAM warm-up: dummy matmuls on a memset scratch run during the
            # DMA dead zone, so real matmuls start at 2.4GHz with no deps
            nc.vector.memset(wscr, 0.0)
            for k in range(16):
                pw = pp_misc.tile([128, NCHUNK], F32, tag="misc", name=f"warm{k}")
                nc.tensor.matmul(pw, wscr[:, 0:128], wscr, start=True, stop=True)
            return st

        def rope_pair(ps_a, ps_b, dest_a, dest_b, cos_c, sinm_c):
            """dest = ps*cos + (R @ ps)*sinm for two head chunks."""
            qq_a = rpool.tile([128, NCHUNK], BF16, tag="qq")
            nc.vector.tensor_copy(qq_a, ps_a)
            qq_b = rpool.tile([128, NCHUNK], BF16, tag="qq2")
            nc.vector.tensor_copy(qq_b, ps_b)
            pr_a = pp_misc.tile([128, NCHUNK], F32, tag="misc")
            nc.tensor.matmul(pr_a, rmat_sb, qq_a, start=True, stop=True)
            pr_b = pp_misc.tile([128, NCHUNK], F32, tag="misc")
            nc.tensor.matmul(pr_b, rmat_sb, qq_b, start=True, stop=True)
            for ps, pr, qq, dest, tg in ((ps_a, pr_a, qq_a, dest_a, "a"),
                                         (ps_b, pr_b, qq_b, dest_b, "b")):
                tcs = rpool.tile([128, NCHUNK], BF16, tag="tcs" + tg, bufs=1)
                nc.vector.tensor_mul(tcs, ps, cos_c)
                # qq is dead after the R-matmul; reuse its slot for the sin product
                nc.vector.tensor_mul(qq, pr, sinm_c)
                nc.vector.tensor_add(dest, tcs, qq)

        def chunk_dmas(qc):
            """Emit the input DMAs for chunk qc; returns chunk state."""
            n0 = qc * NCHUNK
            xsub = []
            for j in range(8):
                t = xpool.tile([128, 4, NCHUNK], BF16, tag="xt", name=f"xt{qc}_{j}")
                nc.gpsimd.dma_start(out=t, in_=xt[qc, j])
                xsub.append(t)
            cos_c = cspool.tile([128, NCHUNK], F32, tag="cos", name=f"cos{qc}")
            nc.gpsimd.dma_start(out=cos_c, in_=cosT[:, n0:n0 + NCHUNK])
            sinm_c = cspool.tile([128, NCHUNK], F32, tag="sinm", name=f"sinm{qc}")
            nc.gpsimd.dma_start(out=sinm_c, in_=sinmT[:, n0:n0 + NCHUNK])
            qt_chunk = qtpool.tile([128, QH, NCHUNK], BF16, tag="qt", name=f"qt{qc}")
            return dict(n0=n0, xsub=xsub, cos=cos_c, sinm=sinm_c, qt=qt_chunk)

        def proj_units(qc, st):
            """13 emission units: 5 feature-group c-loops (+deferred rope), 2 V passes."""
            n0, xsub = st["n0"], st["xsub"]

            def xc(c):
                return xsub[c // 4][:, c % 4, :]

            pend = {}

            def emit_fg(fg, half):
                if half == 0:
                    pend[fg] = (
                        pp_proj.tile([128, NCHUNK], F32, tag="proj", name=f"pa{qc}_{fg}"),
                        pp_proj.tile([128, NCHUNK], F32, tag="proj", name=f"pb{qc}_{fg}"))
                ps_a, ps_b = pend[fg]
                # a/b alternated at cq granularity: the first 4 a-matmuls give
                # the previous fg's rope ~1.7us to drain its banks (bufs=3),
                # and each weight piece is consumed at half the a-then-b rate,
                # which keeps chunk-0 projection behind the HBM delivery curve
                for cq in range(4 * half, 4 * half + 4):
                    for ps, f0 in ((ps_a, 0), (ps_b, 128)):
                        for i in range(4):
                            c = 4 * cq + i
                            nc.tensor.matmul(ps, wt_sb[:, fg, cq, i, f0:f0 + 128],
                                             xc(c), start=(c == 0), stop=(c == CC - 1))

            def emit_rope(fg):
                ps_a, ps_b = pend.pop(fg)
                if fg < 4:
                    d_a = st["qt"][:, 2 * fg, :]
                    d_b = st["qt"][:, 2 * fg + 1, :]
                else:
                    d_a = kt_res[:, 0, n0:n0 + NCHUNK]
                    d_b = kt_res[:, 1, n0:n0 + NCHUNK]
                rope_pair(ps_a, ps_b, d_a, d_b, st["cos"], st["sinm"])

            vps = {}

            def emit_vhalf(p, half):
                if half == 0:
                    vps[p] = [pp_proj.tile([128, 256], F32, tag="proj",
                                           name=f"pv{qc}_{p}_{i}") for i in range(2)]
                ps = vps[p]
                # k outer: ps[1]'s first matmul comes 1.7us after ps[0]'s,
                # giving the previous unit's banks time to drain
                for k in range(2):
                    ns = 2 * p + k
                    for cq in range(4 * half, 4 * half + 4):
                        for i in range(4):
                            c = 4 * cq + i
                            nc.tensor.matmul(ps[k], xc(c)[:, 128 * ns:128 * ns + 128],
                                             wt_sb[:, 5, cq, i, :],
                                             start=(c == 0), stop=(c == CC - 1))
                if half == 1:
                    for k in range(2):
                        ns = 2 * p + k
                        nt_i = 4 * qc + ns
                        nc.vector.tensor_copy(vaug[:, nt_i, :, 0:128],
                                              ps[k].rearrange("p (h w) -> p h w", h=2))

            # K/V first, then the q feature-groups with their ropes streaming
            # out progressively — on the last chunk this lets attention heads
            # interleave into the projection as soon as rope(h//2) lands
            units = [lambda: emit_fg(4, 0), lambda: emit_fg(4, 1),
                     lambda: (emit_rope(4), emit_vhalf(0, 0)),
                     lambda: emit_vhalf(0, 1),
                     lambda: emit_vhalf(1, 0),
                     lambda: emit_vhalf(1, 1),
                     lambda: emit_fg(0, 0), lambda: emit_fg(0, 1)]
            for fg in range(1, 4):
                units.append(lambda fg=fg: (emit_rope(fg - 1), emit_fg(fg, 0)))
                units.append(lambda fg=fg: emit_fg(fg, 1))
            units.append(lambda: emit_rope(3))
            return units

        def attn_units(qc, st):
            """9 emission units: mask DMA + 8 heads; per-s drain as soon as done."""
            n0, qt_chunk = st["n0"], st["qt"]
            nk = 4 * qc + 4
            shared = {}

            def emit_pre():
                m_sb = mpool.tile([128, 4, 128], BF16, tag="mask", name=f"m{qc}")
                nc.gpsimd.dma_start(out=m_sb, in_=maskd[qc])
                shared["m"] = m_sb

            def emit_head(h):
                kv = h // (QH // KVH)
                m_sb = shared["m"]
                pt_tiles = {}
                STAG = 3

                def qk_step(kt):
                    # columns left of q-sub kd are never consumed (per-s AV depth)
                    kd = kt - 4 * qc
                    lo = 128 * kd if kd > 0 else 0
                    ps_s = pp_misc.tile([128, NCHUNK], F32, tag="misc",
                                        name=f"s{qc}_{h}_{kt}")
                    nc.tensor.matmul(ps_s[:, lo:], kt_res[:, kv, 128 * kt:128 * kt + 128],
                                     qt_chunk[:, h, lo:], start=True, stop=True)
                    pt = ptpool.tile([128, NCHUNK], BF16, tag="pt", name=f"pt{qc}_{h}_{kt}")
                    nc.scalar.activation(out=pt[:, lo:], in_=ps_s[:, lo:],
                                         func=mybir.ActivationFunctionType.Exp)
                    if kd >= 0:
                        # causal cut inside the diagonal 128x128 block as a 0/1
                        # multiply on exp'd bf16 in SBUF — off the QK->exp
                        # chain, and the ones-column denominator stays exact
                        # since it accumulates from the zeroed pt
                        nc.vector.tensor_mul(pt[:, 128 * kd:128 * kd + 128],
                                             pt[:, 128 * kd:128 * kd + 128],
                                             m_sb[:, kd, :])
                    pt_tiles[kt] = pt

                # two q-subtiles share one PSUM bank (129-col groups at 0 and
                # 256): accumulation runs start=False onto a DVE-zeroed bank,
                # so the whole-bank has_written clear of start=True never fires
                av_bank = [pp_av.tile([128, 512], F32, tag="av",
                                      name=f"o{qc}_{h}_{p}") for p in range(2)]
                for p in range(2):
                    nc.vector.memset(av_bank[p][:, 0:256 + VW], 0.0)

                def ps_os(s):
                    return av_bank[s // 2][:, 256 * (s % 2):256 * (s % 2) + VW]

                def drain_s(s):
                    po = ps_os(s)
                    den = spool.tile([128, 1], F32, tag="den")
                    nc.vector.reciprocal(den, po[:, 128:129])
                    sc = spool.tile([128, 1], F32, tag="sc")
                    nc.vector.tensor_mul(sc, den, obs_sb[:, 4 * qc + s:4 * qc + s + 1])
                    ob = opool.tile([128, 128], F32, tag="ob", name=f"ob{qc}_{h}_{s}")
                    nc.vector.tensor_scalar_mul(ob, po[:, 0:128], sc)
                    nc.sync.dma_start(out=out[n0 + 128 * s:n0 + 128 * (s + 1),
                                              128 * h:128 * (h + 1)], in_=ob)

                def av_step(kt):
                    pt = pt_tiles.pop(kt)
                    for s in range(4):
                        if kt > 4 * qc + s:
                            continue  # beyond this q-sub's causal depth
                        nc.tensor.matmul(ps_os(s), pt[:, 128 * s:128 * s + 128],
                                         vaug[:, kt, kv, :], start=False,
                                         stop=(kt == 4 * qc + s),
                                         skip_group_check=True)
                    # drain a bank pair only once BOTH its subtiles stopped:
                    # a drain of s0 while s1 still accumulates would force a
                    # PE-behind-DVE serialization on the shared bank
                    if kt == 4 * qc + 1:
                        drain_s(0), drain_s(1)
                    elif kt == 4 * qc + 3:
                        drain_s(2), drain_s(3)

                for kt in range(nk + STAG):
                    if kt < nk:
                        qk_step(kt)
                    if kt >= STAG:
                        av_step(kt - STAG)

            units = [emit_pre]
            for h in range(QH):
                units.append(lambda h=h: emit_head(h))
            return units

        # ---- pipelined emission: attention(qc-1) interleaved with proj(qc) ----
        states = {0: emit_startup()}
        for u in proj_units(0, states[0]):
            u()
        for qc in range(1, NCHUNKS):
            au = attn_units(qc - 1, states[qc - 1])
            states[qc] = chunk_dmas(qc)
            pu = proj_units(qc, states[qc])
            if qc < NCHUNKS - 1:
                # proportional merge of the two unit streams
                tagged = [((i + 0.5) / len(au), 0, u) for i, u in enumerate(au)]
                tagged += [((j + 0.5) / len(pu), 1, u) for j, u in enumerate(pu)]
                for _, _, u in sorted(tagged, key=lambda t: (t[0], t[1])):
                    u()
            else:
                # last round: attention(2) rides proj(3)'s first 9 units, and
                # attention(3) heads follow their rope dependency immediately,
                # so the no-projection tail shrinks to the last head pair
                af = attn_units(qc, states[qc])
                af[0]()                          # chunk-3 mask DMA up front
                stream = []
                for j in range(9):
                    stream.append(au[j])
                    stream.append(pu[j])
                stream += [af[1], pu[9], af[2], pu[10], af[3], pu[11],
                           af[4], pu[12], af[5], pu[13], af[6], pu[14],
                           af[7], af[8]]
                for u in stream:
                    u()
            del states[qc - 1]

    nc.compile()
    return nc


_NC_CACHE = None


def _get_program():
    global _NC_CACHE
    if _NC_CACHE is None:
        _NC_CACHE = _build_program()
    return _NC_CACHE


def _prep_core_inputs(input, weight, cos_cached, sin_cached, attention_mask,
                      position_ids, observation_mask):
    """Build the 8 per-core input maps (host-side shard + layout + bf16 cast)."""
    bf16 = ml_dtypes.bfloat16
    input = np.asarray(input, dtype=np.float32)
    weight = np.asarray(weight, dtype=np.float32)
    cos_cached = np.asarray(cos_cached, dtype=np.float32)
    sin_cached = np.asarray(sin_cached, dtype=np.float32)
    attention_mask = np.asarray(attention_mask, dtype=np.float32)
    position_ids = np.asarray(position_ids)
    observation_mask = np.asarray(observation_mask)

    scale = 1.0 / np.sqrt(HEAD_DIM)
    rmat = np.zeros((128, 128), dtype=np.float32)
    idx = np.arange(128)
    rmat[idx, (idx + 64) % 128] = 1.0
    rmat = rmat.astype(bf16)

    in_maps = []
    for core in range(N_CORES):
        b, g = core // GROUPS, core % GROUPS
        xtT = input[b].T.astype(bf16)                                  # [C, N]
        xt = np.ascontiguousarray(
            xtT.reshape(8, 4, 128, NCHUNKS, NCHUNK).transpose(3, 0, 2, 1, 4))

        wq = weight[g * QH * 128:(g + 1) * QH * 128] * scale           # [1024, C]
        k_off = NUM_HEADS * 128
        wk = weight[k_off + g * KVH * 128:k_off + (g + 1) * KVH * 128]  # [256, C]
        v_off = k_off + KV_HEADS * 128
        wv = weight[v_off + g * KVH * 128:v_off + (g + 1) * KVH * 128]  # [256, C]
        wtT = np.concatenate([wq, wk, wv], axis=0).T.astype(bf16)      # [C, 1536]
        wt = np.ascontiguousarray(
            wtT.reshape(8, 4, 128, 6, 256).transpose(2, 3, 0, 1, 4))   # [128,6,8,4,256]

        pos = position_ids[b]
        cosT = np.ascontiguousarray(cos_cached[0, 0][pos].T)           # [128, N]
        sinmT = np.ascontiguousarray(sin_cached[0, 0][pos].T)
        sinmT[:64] = -sinmT[:64]

        m = attention_mask[b, 0]                                       # [N, N]
        maskd = np.stack([
            np.stack([m[t * 128:(t + 1) * 128, t * 128:(t + 1) * 128].T
                      for t in range(4 * qc, 4 * qc + 4)]).transpose(1, 0, 2)
            for qc in range(NCHUNKS)])                                 # [4, 128, 4, 128]
        maskd = np.ascontiguousarray((maskd == 0.0).astype(bf16))      # 0/1 keep-mask

        obsf = np.ascontiguousarray(
            (observation_mask[b] == 1).astype(np.float32).reshape(NT, 128).T)

        in_maps.append(dict(xt=xt, wt=wt, cosT=cosT, sinmT=sinmT, maskd=maskd,
                            obs=obsf, rmat=rmat))
    return in_maps


def run(inputs: dict, trace: bool = False):
    """Run the sharded kernel; returns (full_output [B*N, C] fp32, BassKernelResults)."""
    nc = _get_program()
    in_maps = _prep_core_inputs(**inputs)
    res = run_bass_kernel_spmd(nc, in_maps, core_ids=list(range(N_CORES)), trace=trace)
    full = np.empty((B, N, C), dtype=np.float32)
    for core in range(N_CORES):
        b, g = core // GROUPS, core % GROUPS
        full[b, :, g * QH * 128:(g + 1) * QH * 128] = res.results[core]["out"]
    return full.reshape(B * N, C), res


def kernel(**inputs) -> np.ndarray:
    out, _ = run(inputs)
    return out


# revision 33
# speedup vs baseline: 1.0102x; 1.0014x over previous
"""Fused QKV+RoPE+GQA-attention kernel for Trainium2, sharded over 8 NeuronCores.

Sharding: data-parallel over batch (B=2), tensor-parallel over heads (4 groups of
8 q-heads / 2 kv-heads per batch element).  Each core computes its output slice
out[b, :, g*1024:(g+1)*1024] independently; no collectives.

Per-core pipeline (n=2048 seq, c=4096 model, d=128 head dim), all matmul
operands in bf16 (fp32 PSUM accumulation; rel-err budget 2e-2 leaves ~20x room):
  - Weights live resident in SBUF (12.6 MB bf16, loaded once) — the fp32r
    predecessor re-streamed all 25 MB of weights from HBM every chunk.
  - QKV^T projection:  Q^T/K^T = W X^T (feature-major, head_dim on partitions),
    V = X W^T (position-major).  bf16 streams 1 col/cycle at any moving dim and
    FWL halves the LDWEIGHTS cost.
  - RoPE without partition shifts: rot_half(q) = sign * (R q) where R is the
    swap-halves permutation applied via one extra matmul per head-chunk, and the
    sign is folded into the sin table on host.
  - Attention: S^T = K_rope Q_rope^T (k on partitions, q on free dim), causal
    structure exploited at 128-tile granularity, the real attention-mask applied
    only inside diagonal 128x128 blocks, softmax without max-subtraction
    (logits are O(10); exp(-3.4e38) = 0 exactly on the ACT LUT), denominator
    from a ones column augmented into V (AV moving dim = 129, per-kv-head
    slice), per-s output drained on the Vector engine as soon as its causal
    depth completes, per-(head,s) 128x128 output DMA on the Sync queue.
  - Emission interleaves attention(chunk i-1) with projection(chunk i): the PE
    executes its queue in order, so alternating independent streams fills each
    stream's dependency bubbles and keeps the HAM clock gate warm.
"""

from contextlib import ExitStack

import numpy as np
import ml_dtypes

import concourse.tile as tile
from concourse import bacc, mybir
from concourse.bass_utils import run_bass_kernel_spmd

F32 = mybir.dt.float32
BF16 = mybir.dt.bfloat16

B, N, C = 2, 2048, 4096
NUM_HEADS, KV_HEADS, HEAD_DIM = 32, 8, 128
GROUPS = 4                     # head groups per batch element
QH = NUM_HEADS // GROUPS       # 8 q heads per core
KVH = KV_HEADS // GROUPS       # 2 kv heads per core
N_CORES = B * GROUPS

NCHUNK = 512                   # seq positions per pass
NCHUNKS = N // NCHUNK          # 4
CC = C // 128                  # 32 contraction chunks
NT = N // 128                  # 16 position tiles
VW = 129                       # AV block width: [v(128) | ones(1)]


def _build_program():
    nc = bacc.Bacc("TRN2", target_bir_lowering=False, debug=False, num_devices=N_CORES)

    xt = nc.dram_tensor("xt", [NCHUNKS, 8, 128, 4, NCHUNK], BF16, kind="ExternalInput").ap()
    wt = nc.dram_tensor("wt", [128, 6, 8, 4, 256], BF16, kind="ExternalInput").ap()
    cosT = nc.dram_tensor("cosT", [128, N], F32, kind="ExternalInput").ap()
    sinmT = nc.dram_tensor("sinmT", [128, N], F32, kind="ExternalInput").ap()
    maskd = nc.dram_tensor("maskd", [NCHUNKS, 128, 4, 128], BF16, kind="ExternalInput").ap()
    obs = nc.dram_tensor("obs", [128, NT], F32, kind="ExternalInput").ap()
    rmat = nc.dram_tensor("rmat", [128, 128], BF16, kind="ExternalInput").ap()
    out = nc.dram_tensor("out", [N, QH * 128], F32, kind="ExternalOutput").ap()

    with tile.TileContext(nc) as tc, ExitStack() as ctx:
        singles = ctx.enter_context(tc.tile_pool(name="singles", bufs=1))
        xpool = ctx.enter_context(tc.tile_pool(name="xt", bufs=9))
        rpool = ctx.enter_context(tc.tile_pool(name="rope", bufs=2))
        cspool = ctx.enter_context(tc.tile_pool(name="cs", bufs=2))
        qtpool = ctx.enter_context(tc.tile_pool(name="qt", bufs=2))
        ptpool = ctx.enter_context(tc.tile_pool(name="pt", bufs=6))
        mpool = ctx.enter_context(tc.tile_pool(name="mask", bufs=2))
        opool = ctx.enter_context(tc.tile_pool(name="outp", bufs=6))
        spool = ctx.enter_context(tc.tile_pool(name="small", bufs=8))
        pp_proj = ctx.enter_context(tc.tile_pool(name="pp_proj", bufs=3, space="PSUM"))
        pp_misc = ctx.enter_context(tc.tile_pool(name="pp_misc", bufs=3, space="PSUM"))
        pp_av = ctx.enter_context(tc.tile_pool(name="pp_av", bufs=2, space="PSUM"))

        # ---- persistent tiles ----
        wt_sb = singles.tile([128, 6, 8, 4, 256], BF16, tag="wt")
        rmat_sb = singles.tile([128, 128], BF16, tag="rmat")
        obs_sb = singles.tile([128, NT], F32, tag="obs")
        kt_res = singles.tile([128, KVH, N], BF16, tag="ktres")      # K^T rope'd
        vaug = singles.tile([128, NT, KVH, VW], BF16, tag="vaug")    # V + ones col

        wscr = singles.tile([128, NCHUNK], BF16, tag="wscr")

        def emit_startup():
            """Chunk-0 + persistent-tile DMAs, deadline-ordered, round-robin
            over all 3 DMA-capable queues (~125 GB/s each).  Interleaving x
            tiles with weight pieces keeps every queue delivering bytes that
            chunk-0 projection (K/V first, then q-fgs) is about to consume."""
            st = dict(n0=0)
            xsub = []
            for j in range(8):
                t = xpool.tile([128, 4, NCHUNK], BF16, tag="xt", name=f"xt0_{j}")
                xsub.append(t)
            st["xsub"] = xsub
            cos_c = cspool.tile([128, NCHUNK], F32, tag="cos", name="cos0")
            sinm_c = cspool.tile([128, NCHUNK], F32, tag="sinm", name="sinm0")
            st["cos"], st["sinm"] = cos_c, sinm_c
            st["qt"] = qtpool.tile([128, QH, NCHUNK], BF16, tag="qt", name="qt0")

            engs = [nc.gpsimd, nc.sync, nc.scalar]
            kq = [0]

            def dma(out, in_):
                engs[kq[0] % 3].dma_start(out=out, in_=in_)
                kq[0] += 1

            def wpiece(fg, c0, w):
                dma(wt_sb[:, fg, c0:c0 + w], wt[:, fg, c0:c0 + w])

            # x tiles front-run the fg4 quarters 2:1 — fg4's c-loop reads
            # xsub[0..7] within its first 13.6us, so x is the tighter deadline
            dma(xsub[0], xt[0, 0])
            wpiece(4, 0, 2)
            dma(xsub[1], xt[0, 1])
            for j in range(3):
                dma(xsub[2 + 2 * j], xt[0, 2 + 2 * j])
                wpiece(4, 2 + 2 * j, 2)
                dma(xsub[3 + 2 * j], xt[0, 3 + 2 * j])
            dma(rmat_sb, rmat)
            dma(cos_c, cosT[:, 0:NCHUNK])
            dma(sinm_c, sinmT[:, 0:NCHUNK])
            for j in range(4):                       # fg5 (V) quarters
                wpiece(5, 2 * j, 2)
            dma(obs_sb, obs)
            for fg in range(4):                      # q-fg halves, in order
                wpiece(fg, 0, 4)
                wpiece(fg, 4, 4)

            nc.vector.memset(vaug[:, :, :, 128:129], 1.0)
            # HAM warm-up: dummy matmuls on a memset scratch run during the
            # DMA dead zone, so real matmuls start at 2.4GHz with no deps
            nc.vector.memset(wscr, 0.0)
            for k in range(28):
                pw = pp_misc.tile([128, NCHUNK], F32, tag="misc", name=f"warm{k}")
                nc.tensor.matmul(pw, wscr[:, 0:128], wscr, start=True, stop=True)
            return st

        def rope_pair(ps_a, ps_b, dest_a, dest_b, cos_c, sinm_c):
            """dest = ps*cos + (R @ ps)*sinm for two head chunks."""
            qq_a = rpool.tile([128, NCHUNK], BF16, tag="qq")
            nc.vector.tensor_copy(qq_a, ps_a)
            qq_b = rpool.tile([128, NCHUNK], BF16, tag="qq2")
            nc.vector.tensor_copy(qq_b, ps_b)
            pr_a = pp_misc.tile([128, NCHUNK], F32, tag="misc")
            nc.tensor.matmul(pr_a, rmat_sb, qq_a, start=True, stop=True)
            pr_b = pp_misc.tile([128, NCHUNK], F32, tag="misc")
            nc.tensor.matmul(pr_b, rmat_sb, qq_b, start=True, stop=True)
            for ps, pr, qq, dest, tg in ((ps_a, pr_a, qq_a, dest_a, "a"),
                                         (ps_b, pr_b, qq_b, dest_b, "b")):
                tcs = rpool.tile([128, NCHUNK], BF16, tag="tcs" + tg, bufs=1)
                nc.vector.tensor_mul(tcs, ps, cos_c)
                # qq is dead after the R-matmul; reuse its slot for the sin product
                nc.vector.tensor_mul(qq, pr, sinm_c)
                nc.vector.tensor_add(dest, tcs, qq)

        def chunk_dmas(qc):
            """Emit the input DMAs for chunk qc; returns chunk state."""
            n0 = qc * NCHUNK
            xsub = []
            for j in range(8):
                t = xpool.tile([128, 4, NCHUNK], BF16, tag="xt", name=f"xt{qc}_{j}")
                nc.gpsimd.dma_start(out=t, in_=xt[qc, j])
                xsub.append(t)
            cos_c = cspool.tile([128, NCHUNK], F32, tag="cos", name=f"cos{qc}")
            nc.gpsimd.dma_start(out=cos_c, in_=cosT[:, n0:n0 + NCHUNK])
            sinm_c = cspool.tile([128, NCHUNK], F32, tag="sinm", name=f"sinm{qc}")
            nc.gpsimd.dma_start(out=sinm_c, in_=sinmT[:, n0:n0 + NCHUNK])
            qt_chunk = qtpool.tile([128, QH, NCHUNK], BF16, tag="qt", name=f"qt{qc}")
            return dict(n0=n0, xsub=xsub, cos=cos_c, sinm=sinm_c, qt=qt_chunk)

        def proj_units(qc, st):
            """13 emission units: 5 feature-group c-loops (+deferred rope), 2 V passes."""
            n0, xsub = st["n0"], st["xsub"]

            def xc(c):
                return xsub[c // 4][:, c % 4, :]

            pend = {}

            def emit_fg(fg, half):
                if half == 0:
                    pend[fg] = (
                        pp_proj.tile([128, NCHUNK], F32, tag="proj", name=f"pa{qc}_{fg}"),
                        pp_proj.tile([128, NCHUNK], F32, tag="proj", name=f"pb{qc}_{fg}"))
                ps_a, ps_b = pend[fg]
                # a/b alternated at cq granularity: the first 4 a-matmuls give
                # the previous fg's rope ~1.7us to drain its banks (bufs=3),
                # and each weight piece is consumed at half the a-then-b rate,
                # which keeps chunk-0 projection behind the HBM delivery curve
                for cq in range(4 * half, 4 * half + 4):
                    for ps, f0 in ((ps_a, 0), (ps_b, 128)):
                        for i in range(4):
                            c = 4 * cq + i
                            nc.tensor.matmul(ps, wt_sb[:, fg, cq, i, f0:f0 + 128],
                                             xc(c), start=(c == 0), stop=(c == CC - 1))

            def emit_rope(fg):
                ps_a, ps_b = pend.pop(fg)
                if fg < 4:
                    d_a = st["qt"][:, 2 * fg, :]
                    d_b = st["qt"][:, 2 * fg + 1, :]
                else:
                    d_a = kt_res[:, 0, n0:n0 + NCHUNK]
                    d_b = kt_res[:, 1, n0:n0 + NCHUNK]
                rope_pair(ps_a, ps_b, d_a, d_b, st["cos"], st["sinm"])

            vps = {}

            def emit_vhalf(p, half):
                if half == 0:
                    vps[p] = [pp_proj.tile([128, 256], F32, tag="proj",
                                           name=f"pv{qc}_{p}_{i}") for i in range(2)]
                ps = vps[p]
                # k outer: ps[1]'s first matmul comes 1.7us after ps[0]'s,
                # giving the previous unit's banks time to drain
                for k in range(2):
                    ns = 2 * p + k
                    for cq in range(4 * half, 4 * half + 4):
                        for i in range(4):
                            c = 4 * cq + i
                            nc.tensor.matmul(ps[k], xc(c)[:, 128 * ns:128 * ns + 128],
                                             wt_sb[:, 5, cq, i, :],
                                             start=(c == 0), stop=(c == CC - 1))
                if half == 1:
                    for k in range(2):
                        ns = 2 * p + k
                        nt_i = 4 * qc + ns
                        nc.vector.tensor_copy(vaug[:, nt_i, :, 0:128],
                                              ps[k].rearrange("p (h w) -> p h w", h=2))

            # K/V first, then the q feature-groups with their ropes streaming
            # out progressively — on the last chunk this lets attention heads
            # interleave into the projection as soon as rope(h//2) lands
            units = [lambda: emit_fg(4, 0), lambda: emit_fg(4, 1),
                     lambda: (emit_rope(4), emit_vhalf(0, 0)),
                     lambda: emit_vhalf(0, 1),
                     lambda: emit_vhalf(1, 0),
                     lambda: emit_vhalf(1, 1),
                     lambda: emit_fg(0, 0), lambda: emit_fg(0, 1)]
            for fg in range(1, 4):
                units.append(lambda fg=fg: (emit_rope(fg - 1), emit_fg(fg, 0)))
                units.append(lambda fg=fg: emit_fg(fg, 1))
            units.append(lambda: emit_rope(3))
            return units

        def attn_units(qc, st):
            """9 emission units: mask DMA + 8 heads; per-s drain as soon as done."""
            n0, qt_chunk = st["n0"], st["qt"]
            nk = 4 * qc + 4
            shared = {}

            def emit_pre():
                m_sb = mpool.tile([128, 4, 128], BF16, tag="mask", name=f"m{qc}")
                nc.gpsimd.dma_start(out=m_sb, in_=maskd[qc])
                shared["m"] = m_sb

            def emit_head(h):
                kv = h // (QH // KVH)
                m_sb = shared["m"]
                pt_tiles = {}
                STAG = 3

                def qk_step(kt):
                    # columns left of q-sub kd are never consumed (per-s AV depth)
                    kd = kt - 4 * qc
                    lo = 128 * kd if kd > 0 else 0
                    ps_s = pp_misc.tile([128, NCHUNK], F32, tag="misc",
                                        name=f"s{qc}_{h}_{kt}")
                    nc.tensor.matmul(ps_s[:, lo:], kt_res[:, kv, 128 * kt:128 * kt + 128],
                                     qt_chunk[:, h, lo:], start=True, stop=True)
                    pt = ptpool.tile([128, NCHUNK], BF16, tag="pt", name=f"pt{qc}_{h}_{kt}")
                    nc.scalar.activation(out=pt[:, lo:], in_=ps_s[:, lo:],
                                         func=mybir.ActivationFunctionType.Exp)
                    if kd >= 0:
                        # causal cut inside the diagonal 128x128 block as a 0/1
                        # multiply on exp'd bf16 in SBUF — off the QK->exp
                        # chain, and the ones-column denominator stays exact
                        # since it accumulates from the zeroed pt
                        nc.vector.tensor_mul(pt[:, 128 * kd:128 * kd + 128],
                                             pt[:, 128 * kd:128 * kd + 128],
                                             m_sb[:, kd, :])
                    pt_tiles[kt] = pt

                # two q-subtiles share one PSUM bank (129-col groups at 0 and
                # 256): accumulation runs start=False onto a DVE-zeroed bank,
                # so the whole-bank has_written clear of start=True never fires
                av_bank = [pp_av.tile([128, 512], F32, tag="av",
                                      name=f"o{qc}_{h}_{p}") for p in range(2)]
                for p in range(2):
                    nc.vector.memset(av_bank[p][:, 0:256 + VW], 0.0)

                def ps_os(s):
                    return av_bank[s // 2][:, 256 * (s % 2):256 * (s % 2) + VW]

                def drain_s(s):
                    po = ps_os(s)
                    den = spool.tile([128, 1], F32, tag="den")
                    nc.vector.reciprocal(den, po[:, 128:129])
                    sc = spool.tile([128, 1], F32, tag="sc")
                    nc.vector.tensor_mul(sc, den, obs_sb[:, 4 * qc + s:4 * qc + s + 1])
                    ob = opool.tile([128, 128], F32, tag="ob", name=f"ob{qc}_{h}_{s}")
                    nc.vector.tensor_scalar_mul(ob, po[:, 0:128], sc)
                    nc.sync.dma_start(out=out[n0 + 128 * s:n0 + 128 * (s + 1),
                                              128 * h:128 * (h + 1)], in_=ob)

                def av_step(kt):
                    pt = pt_tiles.pop(kt)
                    for s in range(4):
                        if kt > 4 * qc + s:
                            continue  # beyond this q-sub's causal depth
                        nc.tensor.matmul(ps_os(s), pt[:, 128 * s:128 * s + 128],
                                         vaug[:, kt, kv, :], start=False,
                                         stop=(kt == 4 * qc + s),
                                         skip_group_check=True)
                    # drain a bank pair only once BOTH its subtiles stopped:
                    # a drain of s0 while s1 still accumulates would force a
                    # PE-behind-DVE serialization on the shared bank
                    if kt == 4 * qc + 1:
                        drain_s(0), drain_s(1)
                    elif kt == 4 * qc + 3:
                        drain_s(2), drain_s(3)

                for kt in range(nk + STAG):
                    if kt < nk:
                        qk_step(kt)
                    if kt >= STAG:
                        av_step(kt - STAG)

            units = [emit_pre]
            for h in range(QH):
                units.append(lambda h=h: emit_head(h))
            return units

        # ---- pipelined emission: attention(qc-1) interleaved with proj(qc) ----
        states = {0: emit_startup()}
        for u in proj_units(0, states[0]):
            u()
        for qc in range(1, NCHUNKS):
            au = attn_units(qc - 1, states[qc - 1])
            states[qc] = chunk_dmas(qc)
            pu = proj_units(qc, states[qc])
            if qc < NCHUNKS - 1:
                # proportional merge of the two unit streams
                tagged = [((i + 0.5) / len(au), 0, u) for i, u in enumerate(au)]
                tagged += [((j + 0.5) / len(pu), 1, u) for j, u in enumerate(pu)]
                for _, _, u in sorted(tagged, key=lambda t: (t[0], t[1])):
                    u()
            else:
                # last round: attention(2) rides proj(3)'s first 9 units, and
                # attention(3) heads follow their rope dependency immediately,
                # so the no-projection tail shrinks to the last head pair
                af = attn_units(qc, states[qc])
                af[0]()                          # chunk-3 mask DMA up front
                stream = []
                for j in range(9):
                    stream.append(au[j])
                    stream.append(pu[j])
                stream += [af[1], pu[9], af[2], pu[10], af[3], pu[11],
                           af[4], pu[12], af[5], pu[13], af[6], pu[14],
                           af[7], af[8]]
                for u in stream:
                    u()
            del states[qc - 1]

    nc.compile()
    return nc


_NC_CACHE = None


def _get_program():
    global _NC_CACHE
    if _NC_CACHE is None:
        _NC_CACHE = _build_program()
    return _NC_CACHE


def _prep_core_inputs(input, weight, cos_cached, sin_cached, attention_mask,
                      position_ids, observation_mask):
    """Build the 8 per-core input maps (host-side shard + layout + bf16 cast)."""
    bf16 = ml_dtypes.bfloat16
    input = np.asarray(input, dtype=np.float32)
    weight = np.asarray(weight, dtype=np.float32)
    cos_cached = np.asarray(cos_cached, dtype=np.float32)
    sin_cached = np.asarray(sin_cached, dtype=np.float32)
    attention_mask = np.asarray(attention_mask, dtype=np.float32)
    position_ids = np.asarray(position_ids)
    observation_mask = np.asarray(observation_mask)

    scale = 1.0 / np.sqrt(HEAD_DIM)
    rmat = np.zeros((128, 128), dtype=np.float32)
    idx = np.arange(128)
    rmat[idx, (idx + 64) % 128] = 1.0
    rmat = rmat.astype(bf16)

    in_maps = []
    for core in range(N_CORES):
        b, g = core // GROUPS, core % GROUPS
        xtT = input[b].T.astype(bf16)                                  # [C, N]
        xt = np.ascontiguousarray(
            xtT.reshape(8, 4, 128, NCHUNKS, NCHUNK).transpose(3, 0, 2, 1, 4))

        wq = weight[g * QH * 128:(g + 1) * QH * 128] * scale           # [1024, C]
        k_off = NUM_HEADS * 128
        wk = weight[k_off + g * KVH * 128:k_off + (g + 1) * KVH * 128]  # [256, C]
        v_off = k_off + KV_HEADS * 128
        wv = weight[v_off + g * KVH * 128:v_off + (g + 1) * KVH * 128]  # [256, C]
        wtT = np.concatenate([wq, wk, wv], axis=0).T.astype(bf16)      # [C, 1536]
        wt = np.ascontiguousarray(
            wtT.reshape(8, 4, 128, 6, 256).transpose(2, 3, 0, 1, 4))   # [128,6,8,4,256]

        pos = position_ids[b]
        cosT = np.ascontiguousarray(cos_cached[0, 0][pos].T)           # [128, N]
        sinmT = np.ascontiguousarray(sin_cached[0, 0][pos].T)
        sinmT[:64] = -sinmT[:64]

        m = attention_mask[b, 0]                                       # [N, N]
        maskd = np.stack([
            np.stack([m[t * 128:(t + 1) * 128, t * 128:(t + 1) * 128].T
                      for t in range(4 * qc, 4 * qc + 4)]).transpose(1, 0, 2)
            for qc in range(NCHUNKS)])                                 # [4, 128, 4, 128]
        maskd = np.ascontiguousarray((maskd == 0.0).astype(bf16))      # 0/1 keep-mask

        obsf = np.ascontiguousarray(
            (observation_mask[b] == 1).astype(np.float32).reshape(NT, 128).T)

        in_maps.append(dict(xt=xt, wt=wt, cosT=cosT, sinmT=sinmT, maskd=maskd,
                            obs=obsf, rmat=rmat))
    return in_maps


def run(inputs: dict, trace: bool = False):
    """Run the sharded kernel; returns (full_output [B*N, C] fp32, BassKernelResults)."""
    nc = _get_program()
    in_maps = _prep_core_inputs(**inputs)
    res = run_bass_kernel_spmd(nc, in_maps, core_ids=list(range(N_CORES)), trace=trace)
    full = np.empty((B, N, C), dtype=np.float32)
    for core in range(N_CORES):
        b, g = core // GROUPS, core % GROUPS
        full[b, :, g * QH * 128:(g + 1) * QH * 128] = res.results[core]["out"]
    return full.reshape(B * N, C), res


def kernel(**inputs) -> np.ndarray:
    out, _ = run(inputs)
    return out


# revision 34
# speedup vs baseline: 1.0111x; 1.0008x over previous
"""Fused QKV+RoPE+GQA-attention kernel for Trainium2, sharded over 8 NeuronCores.

Sharding: data-parallel over batch (B=2), tensor-parallel over heads (4 groups of
8 q-heads / 2 kv-heads per batch element).  Each core computes its output slice
out[b, :, g*1024:(g+1)*1024] independently; no collectives.

Per-core pipeline (n=2048 seq, c=4096 model, d=128 head dim), all matmul
operands in bf16 (fp32 PSUM accumulation; rel-err budget 2e-2 leaves ~20x room):
  - Weights live resident in SBUF (12.6 MB bf16, loaded once) — the fp32r
    predecessor re-streamed all 25 MB of weights from HBM every chunk.
  - QKV^T projection:  Q^T/K^T = W X^T (feature-major, head_dim on partitions),
    V = X W^T (position-major).  bf16 streams 1 col/cycle at any moving dim and
    FWL halves the LDWEIGHTS cost.
  - RoPE without partition shifts: rot_half(q) = sign * (R q) where R is the
    swap-halves permutation applied via one extra matmul per head-chunk, and the
    sign is folded into the sin table on host.
  - Attention: S^T = K_rope Q_rope^T (k on partitions, q on free dim), causal
    structure exploited at 128-tile granularity, the real attention-mask applied
    only inside diagonal 128x128 blocks, softmax without max-subtraction
    (logits are O(10); exp(-3.4e38) = 0 exactly on the ACT LUT), denominator
    from a ones column augmented into V (AV moving dim = 129, per-kv-head
    slice), per-s output drained on the Vector engine as soon as its causal
    depth completes, per-(head,s) 128x128 output DMA on the Sync queue.
  - Emission interleaves attention(chunk i-1) with projection(chunk i): the PE
    executes its queue in order, so alternating independent streams fills each
    stream's dependency bubbles and keeps the HAM clock gate warm.
"""

from contextlib import ExitStack

import numpy as np
import ml_dtypes

import concourse.tile as tile
from concourse import bacc, mybir
from concourse.bass_utils import run_bass_kernel_spmd

F32 = mybir.dt.float32
BF16 = mybir.dt.bfloat16

B, N, C = 2, 2048, 4096
NUM_HEADS, KV_HEADS, HEAD_DIM = 32, 8, 128
GROUPS = 4                     # head groups per batch element
QH = NUM_HEADS // GROUPS       # 8 q heads per core
KVH = KV_HEADS // GROUPS       # 2 kv heads per core
N_CORES = B * GROUPS

NCHUNK = 512                   # seq positions per pass
NCHUNKS = N // NCHUNK          # 4
CC = C // 128                  # 32 contraction chunks
NT = N // 128                  # 16 position tiles
VW = 129                       # AV block width: [v(128) | ones(1)]


def _build_program():
    nc = bacc.Bacc("TRN2", target_bir_lowering=False, debug=False, num_devices=N_CORES)

    xt = nc.dram_tensor("xt", [NCHUNKS, 8, 128, 4, NCHUNK], BF16, kind="ExternalInput").ap()
    wt = nc.dram_tensor("wt", [128, 6, 8, 4, 256], BF16, kind="ExternalInput").ap()
    cosT = nc.dram_tensor("cosT", [128, N], F32, kind="ExternalInput").ap()
    sinmT = nc.dram_tensor("sinmT", [128, N], F32, kind="ExternalInput").ap()
    maskd = nc.dram_tensor("maskd", [NCHUNKS, 128, 4, 128], BF16, kind="ExternalInput").ap()
    obs = nc.dram_tensor("obs", [128, NT], F32, kind="ExternalInput").ap()
    rmat = nc.dram_tensor("rmat", [128, 128], BF16, kind="ExternalInput").ap()
    out = nc.dram_tensor("out", [N, QH * 128], F32, kind="ExternalOutput").ap()

    with tile.TileContext(nc) as tc, ExitStack() as ctx:
        singles = ctx.enter_context(tc.tile_pool(name="singles", bufs=1))
        xpool = ctx.enter_context(tc.tile_pool(name="xt", bufs=9))
        rpool = ctx.enter_context(tc.tile_pool(name="rope", bufs=2))
        cspool = ctx.enter_context(tc.tile_pool(name="cs", bufs=2))
        qtpool = ctx.enter_context(tc.tile_pool(name="qt", bufs=2))
        ptpool = ctx.enter_context(tc.tile_pool(name="pt", bufs=6))
        mpool = ctx.enter_context(tc.tile_pool(name="mask", bufs=2))
        opool = ctx.enter_context(tc.tile_pool(name="outp", bufs=6))
        spool = ctx.enter_context(tc.tile_pool(name="small", bufs=8))
        pp_proj = ctx.enter_context(tc.tile_pool(name="pp_proj", bufs=3, space="PSUM"))
        pp_misc = ctx.enter_context(tc.tile_pool(name="pp_misc", bufs=3, space="PSUM"))
        pp_av = ctx.enter_context(tc.tile_pool(name="pp_av", bufs=2, space="PSUM"))

        # ---- persistent tiles ----
        wt_sb = singles.tile([128, 6, 8, 4, 256], BF16, tag="wt")
        rmat_sb = singles.tile([128, 128], BF16, tag="rmat")
        obs_sb = singles.tile([128, NT], F32, tag="obs")
        kt_res = singles.tile([128, KVH, N], BF16, tag="ktres")      # K^T rope'd
        vaug = singles.tile([128, NT, KVH, VW], BF16, tag="vaug")    # V + ones col

        wscr = singles.tile([128, NCHUNK], BF16, tag="wscr")

        def emit_startup():
            """Chunk-0 + persistent-tile DMAs, deadline-ordered, round-robin
            over all 3 DMA-capable queues (~125 GB/s each).  Interleaving x
            tiles with weight pieces keeps every queue delivering bytes that
            chunk-0 projection (K/V first, then q-fgs) is about to consume."""
            st = dict(n0=0)
            xsub = []
            for j in range(8):
                t = xpool.tile([128, 4, NCHUNK], BF16, tag="xt", name=f"xt0_{j}")
                xsub.append(t)
            st["xsub"] = xsub
            cos_c = cspool.tile([128, NCHUNK], F32, tag="cos", name="cos0")
            sinm_c = cspool.tile([128, NCHUNK], F32, tag="sinm", name="sinm0")
            st["cos"], st["sinm"] = cos_c, sinm_c
            st["qt"] = qtpool.tile([128, QH, NCHUNK], BF16, tag="qt", name="qt0")

            engs = [nc.gpsimd, nc.sync, nc.scalar]
            kq = [0]

            def dma(out, in_):
                engs[kq[0] % 3].dma_start(out=out, in_=in_)
                kq[0] += 1

            def wpiece(fg, c0, w):
                dma(wt_sb[:, fg, c0:c0 + w], wt[:, fg, c0:c0 + w])

            # x tiles front-run the fg4 quarters 2:1 — fg4's c-loop reads
            # xsub[0..7] within its first 13.6us, so x is the tighter deadline
            dma(xsub[0], xt[0, 0])
            wpiece(4, 0, 2)
            dma(xsub[1], xt[0, 1])
            for j in range(3):
                dma(xsub[2 + 2 * j], xt[0, 2 + 2 * j])
                wpiece(4, 2 + 2 * j, 2)
                dma(xsub[3 + 2 * j], xt[0, 3 + 2 * j])
            dma(rmat_sb, rmat)
            wpiece(5, 0, 2)                          # fg5 (V) quarters, first
            dma(cos_c, cosT[:, 0:NCHUNK])            # two ahead of cos/sin
            wpiece(5, 2, 2)
            dma(sinm_c, sinmT[:, 0:NCHUNK])
            wpiece(5, 4, 2)
            wpiece(5, 6, 2)
            dma(obs_sb, obs)
            for fg in range(4):                      # q-fg halves, in order
                wpiece(fg, 0, 4)
                wpiece(fg, 4, 4)

            nc.vector.memset(vaug[:, :, :, 128:129], 1.0)
            # HAM warm-up: dummy matmuls on a memset scratch run during the
            # DMA dead zone, so real matmuls start at 2.4GHz with no deps
            nc.vector.memset(wscr, 0.0)
            for k in range(28):
                pw = pp_misc.tile([128, NCHUNK], F32, tag="misc", name=f"warm{k}")
                nc.tensor.matmul(pw, wscr[:, 0:128], wscr, start=True, stop=True)
            return st

        def rope_pair(ps_a, ps_b, dest_a, dest_b, cos_c, sinm_c):
            """dest = ps*cos + (R @ ps)*sinm for two head chunks."""
            qq_a = rpool.tile([128, NCHUNK], BF16, tag="qq")
            nc.vector.tensor_copy(qq_a, ps_a)
            qq_b = rpool.tile([128, NCHUNK], BF16, tag="qq2")
            nc.vector.tensor_copy(qq_b, ps_b)
            pr_a = pp_misc.tile([128, NCHUNK], F32, tag="misc")
            nc.tensor.matmul(pr_a, rmat_sb, qq_a, start=True, stop=True)
            pr_b = pp_misc.tile([128, NCHUNK], F32, tag="misc")
            nc.tensor.matmul(pr_b, rmat_sb, qq_b, start=True, stop=True)
            for ps, pr, qq, dest, tg in ((ps_a, pr_a, qq_a, dest_a, "a"),
                                         (ps_b, pr_b, qq_b, dest_b, "b")):
                tcs = rpool.tile([128, NCHUNK], BF16, tag="tcs" + tg, bufs=1)
                nc.vector.tensor_mul(tcs, ps, cos_c)
                # qq is dead after the R-matmul; reuse its slot for the sin product
                nc.vector.tensor_mul(qq, pr, sinm_c)
                nc.vector.tensor_add(dest, tcs, qq)

        def chunk_dmas(qc):
            """Emit the input DMAs for chunk qc; returns chunk state."""
            n0 = qc * NCHUNK
            xsub = []
            for j in range(8):
                t = xpool.tile([128, 4, NCHUNK], BF16, tag="xt", name=f"xt{qc}_{j}")
                nc.gpsimd.dma_start(out=t, in_=xt[qc, j])
                xsub.append(t)
            cos_c = cspool.tile([128, NCHUNK], F32, tag="cos", name=f"cos{qc}")
            nc.gpsimd.dma_start(out=cos_c, in_=cosT[:, n0:n0 + NCHUNK])
            sinm_c = cspool.tile([128, NCHUNK], F32, tag="sinm", name=f"sinm{qc}")
            nc.gpsimd.dma_start(out=sinm_c, in_=sinmT[:, n0:n0 + NCHUNK])
            qt_chunk = qtpool.tile([128, QH, NCHUNK], BF16, tag="qt", name=f"qt{qc}")
            return dict(n0=n0, xsub=xsub, cos=cos_c, sinm=sinm_c, qt=qt_chunk)

        def proj_units(qc, st):
            """13 emission units: 5 feature-group c-loops (+deferred rope), 2 V passes."""
            n0, xsub = st["n0"], st["xsub"]

            def xc(c):
                return xsub[c // 4][:, c % 4, :]

            pend = {}

            def emit_fg(fg, half):
                if half == 0:
                    pend[fg] = (
                        pp_proj.tile([128, NCHUNK], F32, tag="proj", name=f"pa{qc}_{fg}"),
                        pp_proj.tile([128, NCHUNK], F32, tag="proj", name=f"pb{qc}_{fg}"))
                ps_a, ps_b = pend[fg]
                # a/b alternated at cq granularity: the first 4 a-matmuls give
                # the previous fg's rope ~1.7us to drain its banks (bufs=3),
                # and each weight piece is consumed at half the a-then-b rate,
                # which keeps chunk-0 projection behind the HBM delivery curve
                for cq in range(4 * half, 4 * half + 4):
                    for ps, f0 in ((ps_a, 0), (ps_b, 128)):
                        for i in range(4):
                            c = 4 * cq + i
                            nc.tensor.matmul(ps, wt_sb[:, fg, cq, i, f0:f0 + 128],
                                             xc(c), start=(c == 0), stop=(c == CC - 1))

            def emit_rope(fg):
                ps_a, ps_b = pend.pop(fg)
                if fg < 4:
                    d_a = st["qt"][:, 2 * fg, :]
                    d_b = st["qt"][:, 2 * fg + 1, :]
                else:
                    d_a = kt_res[:, 0, n0:n0 + NCHUNK]
                    d_b = kt_res[:, 1, n0:n0 + NCHUNK]
                rope_pair(ps_a, ps_b, d_a, d_b, st["cos"], st["sinm"])

            vps = {}

            def emit_vhalf(p, half):
                if half == 0:
                    vps[p] = [pp_proj.tile([128, 256], F32, tag="proj",
                                           name=f"pv{qc}_{p}_{i}") for i in range(2)]
                ps = vps[p]
                # k outer: ps[1]'s first matmul comes 1.7us after ps[0]'s,
                # giving the previous unit's banks time to drain
                for k in range(2):
                    ns = 2 * p + k
                    for cq in range(4 * half, 4 * half + 4):
                        for i in range(4):
                            c = 4 * cq + i
                            nc.tensor.matmul(ps[k], xc(c)[:, 128 * ns:128 * ns + 128],
                                             wt_sb[:, 5, cq, i, :],
                                             start=(c == 0), stop=(c == CC - 1))
                if half == 1:
                    for k in range(2):
                        ns = 2 * p + k
                        nt_i = 4 * qc + ns
                        nc.vector.tensor_copy(vaug[:, nt_i, :, 0:128],
                                              ps[k].rearrange("p (h w) -> p h w", h=2))

            # K/V first, then the q feature-groups with their ropes streaming
            # out progressively — on the last chunk this lets attention heads
            # interleave into the projection as soon as rope(h//2) lands
            units = [lambda: emit_fg(4, 0), lambda: emit_fg(4, 1),
                     lambda: (emit_rope(4), emit_vhalf(0, 0)),
                     lambda: emit_vhalf(0, 1),
                     lambda: emit_vhalf(1, 0),
                     lambda: emit_vhalf(1, 1),
                     lambda: emit_fg(0, 0), lambda: emit_fg(0, 1)]
            for fg in range(1, 4):
                units.append(lambda fg=fg: (emit_rope(fg - 1), emit_fg(fg, 0)))
                units.append(lambda fg=fg: emit_fg(fg, 1))
            units.append(lambda: emit_rope(3))
            return units

        def attn_units(qc, st):
            """9 emission units: mask DMA + 8 heads; per-s drain as soon as done."""
            n0, qt_chunk = st["n0"], st["qt"]
            nk = 4 * qc + 4
            shared = {}

            def emit_pre():
                m_sb = mpool.tile([128, 4, 128], BF16, tag="mask", name=f"m{qc}")
                nc.gpsimd.dma_start(out=m_sb, in_=maskd[qc])
                shared["m"] = m_sb

            def emit_head(h):
                kv = h // (QH // KVH)
                m_sb = shared["m"]
                pt_tiles = {}
                STAG = 3

                def qk_step(kt):
                    # columns left of q-sub kd are never consumed (per-s AV depth)
                    kd = kt - 4 * qc
                    lo = 128 * kd if kd > 0 else 0
                    ps_s = pp_misc.tile([128, NCHUNK], F32, tag="misc",
                                        name=f"s{qc}_{h}_{kt}")
                    nc.tensor.matmul(ps_s[:, lo:], kt_res[:, kv, 128 * kt:128 * kt + 128],
                                     qt_chunk[:, h, lo:], start=True, stop=True)
                    pt = ptpool.tile([128, NCHUNK], BF16, tag="pt", name=f"pt{qc}_{h}_{kt}")
                    nc.scalar.activation(out=pt[:, lo:], in_=ps_s[:, lo:],
                                         func=mybir.ActivationFunctionType.Exp)
                    if kd >= 0:
                        # causal cut inside the diagonal 128x128 block as a 0/1
                        # multiply on exp'd bf16 in SBUF — off the QK->exp
                        # chain, and the ones-column denominator stays exact
                        # since it accumulates from the zeroed pt
                        nc.vector.tensor_mul(pt[:, 128 * kd:128 * kd + 128],
                                             pt[:, 128 * kd:128 * kd + 128],
                                             m_sb[:, kd, :])
                    pt_tiles[kt] = pt

                # two q-subtiles share one PSUM bank (129-col groups at 0 and
                # 256): accumulation runs start=False onto a DVE-zeroed bank,
                # so the whole-bank has_written clear of start=True never fires
                av_bank = [pp_av.tile([128, 512], F32, tag="av",
                                      name=f"o{qc}_{h}_{p}") for p in range(2)]
                for p in range(2):
                    nc.vector.memset(av_bank[p][:, 0:256 + VW], 0.0)

                def ps_os(s):
                    return av_bank[s // 2][:, 256 * (s % 2):256 * (s % 2) + VW]

                def drain_s(s):
                    po = ps_os(s)
                    den = spool.tile([128, 1], F32, tag="den")
                    nc.vector.reciprocal(den, po[:, 128:129])
                    sc = spool.tile([128, 1], F32, tag="sc")
                    nc.vector.tensor_mul(sc, den, obs_sb[:, 4 * qc + s:4 * qc + s + 1])
                    ob = opool.tile([128, 128], F32, tag="ob", name=f"ob{qc}_{h}_{s}")
                    nc.vector.tensor_scalar_mul(ob, po[:, 0:128], sc)
                    nc.sync.dma_start(out=out[n0 + 128 * s:n0 + 128 * (s + 1),
                                              128 * h:128 * (h + 1)], in_=ob)

                def av_step(kt):
                    pt = pt_tiles.pop(kt)
                    for s in range(4):
                        if kt > 4 * qc + s:
                            continue  # beyond this q-sub's causal depth
                        nc.tensor.matmul(ps_os(s), pt[:, 128 * s:128 * s + 128],
                                         vaug[:, kt, kv, :], start=False,
                                         stop=(kt == 4 * qc + s),
                                         skip_group_check=True)
                    # drain a bank pair only once BOTH its subtiles stopped:
                    # a drain of s0 while s1 still accumulates would force a
                    # PE-behind-DVE serialization on the shared bank
                    if kt == 4 * qc + 1:
                        drain_s(0), drain_s(1)
                    elif kt == 4 * qc + 3:
                        drain_s(2), drain_s(3)

                for kt in range(nk + STAG):
                    if kt < nk:
                        qk_step(kt)
                    if kt >= STAG:
                        av_step(kt - STAG)

            units = [emit_pre]
            for h in range(QH):
                units.append(lambda h=h: emit_head(h))
            return units

        # ---- pipelined emission: attention(qc-1) interleaved with proj(qc) ----
        states = {0: emit_startup()}
        for u in proj_units(0, states[0]):
            u()
        for qc in range(1, NCHUNKS):
            au = attn_units(qc - 1, states[qc - 1])
            states[qc] = chunk_dmas(qc)
            pu = proj_units(qc, states[qc])
            if qc < NCHUNKS - 1:
                # proportional merge of the two unit streams
                tagged = [((i + 0.5) / len(au), 0, u) for i, u in enumerate(au)]
                tagged += [((j + 0.5) / len(pu), 1, u) for j, u in enumerate(pu)]
                for _, _, u in sorted(tagged, key=lambda t: (t[0], t[1])):
                    u()
            else:
                # last round: attention(2) rides proj(3)'s first 9 units, and
                # attention(3) heads follow their rope dependency immediately,
                # so the no-projection tail shrinks to the last head pair
                af = attn_units(qc, states[qc])
                af[0]()                          # chunk-3 mask DMA up front
                stream = []
                for j in range(9):
                    stream.append(au[j])
                    stream.append(pu[j])
                stream += [af[1], pu[9], af[2], pu[10], af[3], pu[11],
                           af[4], pu[12], af[5], pu[13], af[6], pu[14],
                           af[7], af[8]]
                for u in stream:
                    u()
            del states[qc - 1]

    nc.compile()
    return nc


_NC_CACHE = None


def _get_program():
    global _NC_CACHE
    if _NC_CACHE is None:
        _NC_CACHE = _build_program()
    return _NC_CACHE


def _prep_core_inputs(input, weight, cos_cached, sin_cached, attention_mask,
                      position_ids, observation_mask):
    """Build the 8 per-core input maps (host-side shard + layout + bf16 cast)."""
    bf16 = ml_dtypes.bfloat16
    input = np.asarray(input, dtype=np.float32)
    weight = np.asarray(weight, dtype=np.float32)
    cos_cached = np.asarray(cos_cached, dtype=np.float32)
    sin_cached = np.asarray(sin_cached, dtype=np.float32)
    attention_mask = np.asarray(attention_mask, dtype=np.float32)
    position_ids = np.asarray(position_ids)
    observation_mask = np.asarray(observation_mask)

    scale = 1.0 / np.sqrt(HEAD_DIM)
    rmat = np.zeros((128, 128), dtype=np.float32)
    idx = np.arange(128)
    rmat[idx, (idx + 64) % 128] = 1.0
    rmat = rmat.astype(bf16)

    in_maps = []
    for core in range(N_CORES):
        b, g = core // GROUPS, core % GROUPS
        xtT = input[b].T.astype(bf16)                                  # [C, N]
        xt = np.ascontiguousarray(
            xtT.reshape(8, 4, 128, NCHUNKS, NCHUNK).transpose(3, 0, 2, 1, 4))

        wq = weight[g * QH * 128:(g + 1) * QH * 128] * scale           # [1024, C]
        k_off = NUM_HEADS * 128
        wk = weight[k_off + g * KVH * 128:k_off + (g + 1) * KVH * 128]  # [256, C]
        v_off = k_off + KV_HEADS * 128
        wv = weight[v_off + g * KVH * 128:v_off + (g + 1) * KVH * 128]  # [256, C]
        wtT = np.concatenate([wq, wk, wv], axis=0).T.astype(bf16)      # [C, 1536]
        wt = np.ascontiguousarray(
            wtT.reshape(8, 4, 128, 6, 256).transpose(2, 3, 0, 1, 4))   # [128,6,8,4,256]

        pos = position_ids[b]
        cosT = np.ascontiguousarray(cos_cached[0, 0][pos].T)           # [128, N]
        sinmT = np.ascontiguousarray(sin_cached[0, 0][pos].T)
        sinmT[:64] = -sinmT[:64]

        m = attention_mask[b, 0]                                       # [N, N]
        maskd = np.stack([
            np.stack([m[t * 128:(t + 1) * 128, t * 128:(t + 1) * 128].T
                      for t in range(4 * qc, 4 * qc + 4)]).transpose(1, 0, 2)
            for qc in range(NCHUNKS)])                                 # [4, 128, 4, 128]
        maskd = np.ascontiguousarray((maskd == 0.0).astype(bf16))      # 0/1 keep-mask

        obsf = np.ascontiguousarray(
            (observation_mask[b] == 1).astype(np.float32).reshape(NT, 128).T)

        in_maps.append(dict(xt=xt, wt=wt, cosT=cosT, sinmT=sinmT, maskd=maskd,
                            obs=obsf, rmat=rmat))
    return in_maps


def run(inputs: dict, trace: bool = False):
    """Run the sharded kernel; returns (full_output [B*N, C] fp32, BassKernelResults)."""
    nc = _get_program()
    in_maps = _prep_core_inputs(**inputs)
    res = run_bass_kernel_spmd(nc, in_maps, core_ids=list(range(N_CORES)), trace=trace)
    full = np.empty((B, N, C), dtype=np.float32)
    for core in range(N_CORES):
        b, g = core // GROUPS, core % GROUPS
        full[b, :, g * QH * 128:(g + 1) * QH * 128] = res.results[core]["out"]
    return full.reshape(B * N, C), res


def kernel(**inputs) -> np.ndarray:
    out, _ = run(inputs)
    return out
